# revision 1
# baseline (speedup 1.0000x reference)
"""Trainium2 Bass kernel v2 for nn_Attention_9921374454177.

RMSNorm -> QKV proj -> 16-head causal attention -> out proj.
Sharding: 8 cores = 2 batches x 4 head-groups; host sums the 4 partial
out-projections per batch.

v2 changes vs baseline (tuned against the TimelineSim cost model):
  - QKV projections run as fp8 DoubleRow matmuls with hi+lo error
    compensation: x and w are shipped as (e4m3 hi, e5m2 residual) pairs and
    the projection accumulates the 3 significant cross terms
    (hi*hi + hi*lo + lo*hi) in one PSUM group. 0.75x the PE cost of bf16 at
    ~bf16 accuracy. Weights are pre-scaled x16 on host (fp8 subnormal
    avoidance); the descale rides the existing s_b/s_pp token-scale folds.
  - sum-of-squares ships x^2 as e4m3 from host and reduces it with an fp8
    DoubleRow ones-matmul (4x cheaper than the bf16 ones-trick, and no ACT
    Square pass at all).
  - S, PV, and the out-projection run in bf16 (fp8 at these sites measures
    1.8-3.5e-2 final rel err -- over the 2e-2 gate), with exact causal
    widths per j-tile (bf16 has no 256-min-width penalty like fp32r).
  - causal mask added by the PE as an fp8 DoubleRow rank-structured matmul
    (strict-lower-tri(-60) @ identity window) into the same PSUM group.
  - mask folded into the v token scale and the l ones-column (exact masking
    through the PV matmul) -> exp needs no bias/scale AP at all; k-norm
    scale folded into the k PSUM->bf16 cast instead of the exp scale.
  - exp batched over both heads of a pair (one ACT op per (m, jt)).
  - attention emits one unified (m, jt) stream with a lag-1 exp pipeline;
    out-projection of earlier blocks and k/q/v projection of the next block
    are interleaved as paced filler units between steps (with deferral so
    the later, larger i-batches get enough independent PE work), and the
    next pair's S matmuls overlap the previous pair's normalization chain.
  - causal-mask add truncated to the 128 columns it can affect; startup
    DMAs blob-packed (u8 + bitcast views) and ordered by first consumer.
  - note: GPSIMD ops and the custom-DVE reciprocal cannot read PSUM on
    hardware -- every PSUM consumer here is a plain DVE/ACT op or the PE.
"""
import numpy as np
import ml_dtypes

import concourse.bacc as bacc
import concourse.mybir as mybir
import concourse.tile as tile
from concourse.bass_utils import run_bass_kernel_spmd

F32 = mybir.dt.float32
BF16 = mybir.dt.bfloat16
F8H = mybir.dt.float8e4   # e4m3
F8L = mybir.dt.float8e5   # e5m2
AF = mybir.ActivationFunctionType
OP = mybir.AluOpType
DR = mybir.MatmulPerfMode.DoubleRow

NP_F8H = ml_dtypes.float8_e4m3
NP_F8L = ml_dtypes.float8_e5m2
NP_BF16 = ml_dtypes.bfloat16

B, N, DIM = 2, 2048, 1024
HEADS, DHEAD = 16, 64
GH = 4                 # heads per core
GF = GH * DHEAD        # 256 features per core
NCORES = 8
TBS = 512              # token block size
NTB = N // TBS         # 4
NJT = N // 128         # 16 j-tiles
WS = 16.0              # host weight pre-scale (fp8 subnormal avoidance)
LN2 = float(np.log(2.0))   # exp bias: s_b = 2*ss^-1/2 = (s/16 with s=32*ss^-1/2)

_COMBINED_ACT_SET = "natural_log_exp_and_others"


class _Bacc(bacc.Bacc):
    """Pin the ln+exp combined ACT table so Ln/Exp share one table load."""

    def insert_act_table_loads(self):
        import bass_rust as _bass_rust
        from concourse.hw_specs import get_activation_tables

        has_activation = any(
            isinstance(i, mybir.InstActivation)
            for b in self.main_func.blocks
            for i in b.instructions
        )
        if not has_activation:
            return
        tables = [
            (name, funcs if name == _COMBINED_ACT_SET else set())
            for name, funcs in get_activation_tables(self.m.arch).items()
        ]
        _bass_rust.insert_act_table_loads(self, tables)


def _build():
    nc = _Bacc()
    U8 = mybir.dt.uint8
    xhl = nc.declare_dram_parameter("xhl", [128, 2, NTB, 8, TBS], U8, isOutput=False)
    xq8 = nc.declare_dram_parameter("xq8", [128, 8, N], F8H, isOutput=False)
    wblob = nc.declare_dram_parameter("wblob", [128, 6, 8 * GF], U8, isOutput=False)
    wo = nc.declare_dram_parameter("wo", [128, 2, DIM], BF16, isOutput=False)
    c8blob = nc.declare_dram_parameter("c8blob", [128, 2, 768], U8, isOutput=False)
    maskv = nc.declare_dram_parameter("maskv", [128, NJT], F32, isOutput=False)
    idnb = nc.declare_dram_parameter("idnb", [128, 128], F32, isOutput=False)
    out = nc.declare_dram_parameter("out", [N, DIM], BF16, isOutput=True)

    with tile.TileContext(nc) as tc:
        with (
            tc.tile_pool(name="const", bufs=1) as cp,
            tc.tile_pool(name="xsl", bufs=2) as xp,
            tc.tile_pool(name="sm", bufs=1) as smp,
            tc.tile_pool(name="pTp", bufs=4) as pp,
            tc.tile_pool(name="lstp", bufs=1) as lp,
            tc.tile_pool(name="bcp", bufs=1) as bp,
            tc.tile_pool(name="O2p", bufs=4) as o2p,
            tc.tile_pool(name="ps", bufs=8, space="PSUM") as ps,
        ):
            # ---- startup DMAs, ordered by first consumer: c8 consts + xq
            # (ss-DR), wk + x chunk-pairs (k-proj), then the rest.
            c8_t = cp.tile([128, 2, 768], mybir.dt.uint8, name="c8_t")
            nc.sync.dma_start(c8_t[:], c8blob[:])
            ones_t = c8_t[:, :, 0:128].bitcast(F8H)
            tri8_t = c8_t[:, :, 128:256].bitcast(F8H)
            sel8_t = c8_t[:, :, 256:768].bitcast(F8H)
            # touch ACT so the single activation-table load runs during the
            # prologue DMAs, off the first Ln's critical path
            actwarm = cp.tile([128, 1], F32, name="actwarm")
            nc.vector.memset(actwarm[:], 1.0)
            nc.scalar.activation(actwarm[:], actwarm[:], AF.Exp)

            xq0 = xp.tile([128, 8, TBS], F8H, name="xq0", tag="xq")
            nc.sync.dma_start(xq0[:], xq8[:, :, 0:TBS])
            wb_t = cp.tile([128, 6, 8 * GF], mybir.dt.uint8, name="wb_t")
            nc.sync.dma_start(wb_t[:, 0:2, :], wblob[:, 0:2, :])

            def wview(i, dt_):
                return wb_t[:, i, :].rearrange("p (c f) -> p c f", f=GF).bitcast(dt_)

            wkh_t, wkl_t = wview(0, F8H), wview(1, F8L)
            wqh_t, wql_t = wview(2, F8H), wview(3, F8L)
            wvh_t, wvl_t = wview(4, F8H), wview(5, F8L)

            # x hi+lo for block 0 in 4 chunk-pair pieces so the first
            # k-projection group unblocks incrementally
            xt0 = xp.tile([128, 2, 8, TBS], mybir.dt.uint8, name="xt0", tag="xhl")
            for c0 in (0, 4):
                for hl in range(2):
                    nc.sync.dma_start(xt0[:, hl, c0:c0 + 4, :],
                                      xhl[:, hl, 0, c0:c0 + 4, :])
            xh0 = xt0[:, 0, :, :].bitcast(F8H)
            xl0 = xt0[:, 1, :, :].bitcast(F8L)

            nc.sync.dma_start(wb_t[:, 2:4, :], wblob[:, 2:4, :])
            nc.sync.dma_start(wb_t[:, 4:6, :], wblob[:, 4:6, :])
            maskv_t = cp.tile([128, NJT], F32, name="maskv_t")
            nc.sync.dma_start(maskv_t[:], maskv[:])
            idnb_t = cp.tile([128, 128], F32, name="idnb_t")
            nc.sync.dma_start(idnb_t[:], idnb[:])
            wo_t = cp.tile([128, 2, DIM], BF16, name="wo_t")

            ln2_t = cp.tile([128, 1], F32, name="ln2_t")
            nc.vector.memset(ln2_t[:], LN2)
            ones64_t = cp.tile([128, 64], F32, name="ones64_t")
            nc.vector.memset(ones64_t[:], 1.0)

            # ---- persistent activation tensors ----
            v_sb = cp.tile([128, NJT, GH, DHEAD + 1], BF16, name="v_sb")
            # l ones-column <- mask (exact masking through the PV matmul)
            for h in range(GH):
                nc.vector.tensor_copy(
                    v_sb[:, :, h, DHEAD:DHEAD + 1].rearrange("p a b -> p (a b)"),
                    maskv_t[:])
            kT = [cp.tile([128, N], BF16, name=f"kT{ft}") for ft in range(2)]
            qT = [cp.tile([128, N], BF16, name=f"qT{ft}") for ft in range(2)]
            s_b = [cp.tile([128, TBS], F32, name=f"s_b{tb}") for tb in range(NTB)]
            s_pv = cp.tile([128, NJT], F32, name="s_pv")

            o2_of = {}

            def dr3(out_ap, lhs_hl, rhs_hl, lhs_sl, rhs_sl, first, last):
                """One 3-term hi+lo DoubleRow contraction step into out_ap.
                lhs_hl/rhs_hl = (hi_tile, lo_tile); *_sl = slicer functions."""
                lh, ll = lhs_hl
                rh, rl = rhs_hl
                terms = [(lh, rh), (lh, rl), (ll, rh)]
                for i, (lt, rt) in enumerate(terms):
                    nc.tensor.matmul(out_ap, lhs_sl(lt), rhs_sl(rt),
                                     start=(first and i == 0),
                                     stop=(last and i == 2),
                                     perf_mode=DR)

            def rms_chain(tb, xq):
                """sum-of-squares (fp8-DR ones-trick) -> s_b[tb] = 2*ss^-0.5."""
                ss_ps = ps.tile([128, TBS], F32, name="ss_ps", tag="ps", bufs=2)
                for th in range(2):
                    for cp_ in range(4):
                        nc.tensor.matmul(
                            ss_ps[:, th * 256:(th + 1) * 256],
                            ones_t[:],
                            xq[:, 2 * cp_:2 * cp_ + 2, th * 256:(th + 1) * 256],
                            start=(cp_ == 0), stop=(cp_ == 3), perf_mode=DR)
                lnt = smp.tile([128, TBS], F32, name="lnt", tag="lnt")
                nc.scalar.activation(lnt[:], ss_ps[:], AF.Ln)
                s0 = smp.tile([128, TBS], F32, name="s0", tag="s0")
                nc.scalar.activation(s0[:], lnt[:], AF.Exp, scale=-0.5, bias=ln2_t[:])
                u_t = smp.tile([128, TBS], F32, name="u_t", tag="u_t")
                nc.vector.tensor_mul(u_t[:], s0[:], s0[:])
                w_t = smp.tile([128, TBS], F32, name="w_t", tag="w_t")
                nc.vector.tensor_mul(w_t[:], u_t[:], ss_ps[:])
                nc.vector.tensor_scalar(w_t[:], w_t[:], -0.125, 1.5, OP.mult, OP.add)
                nc.vector.tensor_mul(s_b[tb][:], s0[:], w_t[:])

            def phase1B_units(tb, xh, xl):
                """Yield v-projection steps for block tb (transpose + scale
                fold first, then 4 fp8-DR 3-term quarter-blocks)."""
                t0 = tb * TBS

                def unit_t():
                    tps = ps.tile([128, TBS], F32, name="tps", tag="ps", bufs=2)
                    for j in range(4):
                        nc.tensor.transpose(tps[:, j * 128:(j + 1) * 128],
                                            s_b[tb][:, j * 128:(j + 1) * 128], idnb_t[:])
                    s_pp_blk = smp.tile([128, 4], F32, name="s_pp_blk", tag="spb")
                    nc.vector.tensor_copy(
                        s_pp_blk[:],
                        tps[:].rearrange("p (j q) -> p j q", q=128)[:, :, 0:1]
                            .rearrange("p j q -> p (j q)"))
                    nc.vector.tensor_mul(s_pv[:, tb * 4:(tb + 1) * 4], s_pp_blk[:],
                                         maskv_t[:, tb * 4:(tb + 1) * 4])

                yield unit_t
                for tsub in range(4):
                    def unit_v(tsub=tsub):
                        vps = ps.tile([128, GF], F32, name="vps", tag="ps", bufs=2)
                        for cp_ in range(4):
                            dr3(vps[:],
                                (xh, xl), (wvh_t, wvl_t),
                                lambda t, c=cp_, ts=tsub: t[:, 2 * c:2 * c + 2, ts * 128:(ts + 1) * 128],
                                lambda t, c=cp_: t[:, 2 * c:2 * c + 2, :],
                                first=(cp_ == 0), last=(cp_ == 3))
                        t_idx = tb * 4 + tsub
                        nc.vector.tensor_scalar_mul(
                            v_sb[:, t_idx, :, 0:DHEAD],
                            vps[:].rearrange("p (h d) -> p h d", d=DHEAD),
                            s_pv[:, t_idx:t_idx + 1])

                    yield unit_v

            def norm_pair(ib, m, o_ps, tail):
                """1/l + normalization for head pair m of i-batch ib."""
                O2m = o2p.tile([128, TBS], BF16, name=f"O2_{m}", tag="O2", bufs=6)
                o2_of[(ib, m)] = O2m
                lst = lp.tile([1, 2 * TBS], F32, name="lst", tag="lst", bufs=2)
                rcl = lp.tile([1, 2 * TBS], F32, name="rcl", tag="rcl", bufs=2)
                if tail:
                    # chunked 128-col chains (copy on idle ACT, recip+mul on
                    # DVE, bcast on Pool) so outproj's stationary loads
                    # unblock it-by-it right behind the last PV
                    bchs = [bp.tile([64, TBS], F32, name=f"bch{h2}", tag="bch",
                                    bufs=2) for h2 in range(2)]
                    for hf in range(2):
                        cs = slice(hf * 256, (hf + 1) * 256)
                        for h2 in range(2):
                            lsl = slice(h2 * TBS + hf * 256, h2 * TBS + (hf + 1) * 256)
                            nc.scalar.activation(lst[0:1, lsl],
                                                 o_ps[h2][64:65, cs], AF.Identity)
                            nc.vector.reciprocal_approx_fast(
                                out=rcl[0:1, lsl], in_=lst[0:1, lsl])
                            nc.gpsimd.partition_broadcast(
                                bchs[h2][:, cs], rcl[0:1, lsl])
                            nc.vector.tensor_mul(
                                O2m[h2 * 64:(h2 + 1) * 64, cs],
                                o_ps[h2][0:DHEAD, cs], bchs[h2][:, cs])
                else:
                    for h2 in range(2):
                        nc.vector.tensor_copy(lst[0:1, h2 * TBS:(h2 + 1) * TBS],
                                              o_ps[h2][64:65, :])
                        nc.vector.reciprocal_approx_fast(
                            out=rcl[0:1, h2 * TBS:(h2 + 1) * TBS],
                            in_=lst[0:1, h2 * TBS:(h2 + 1) * TBS])
                        bch = bp.tile([64, TBS], F32, name=f"bch{h2}", tag="bch", bufs=2)
                        nc.gpsimd.partition_broadcast(
                            bch[:], rcl[0:1, h2 * TBS:(h2 + 1) * TBS])
                        nc.vector.tensor_mul(O2m[h2 * 64:(h2 + 1) * 64, :],
                                             o_ps[h2][0:DHEAD, :], bch[:])

            def outproj_units(ib):
                """Yield fine-grained outproj steps; one out-DMA per
                128-token row tile (both oc halves share one ost tile)."""
                i0 = ib * TBS
                holders = [dict() for _ in range(4)]
                for it in range(4):
                    for oc in range(2):
                        def unit(it=it, oc=oc):
                            opps = ps.tile([128, TBS], F32, name="opps", tag="ps", bufs=2)
                            for m in range(2):
                                nc.tensor.matmul(opps[:],
                                                 o2_of[(ib, m)][:, it * 128:(it + 1) * 128],
                                                 wo_t[:, m, oc * 512:(oc + 1) * 512],
                                                 start=(m == 0), stop=(m == 1))
                            if oc == 0:
                                holders[it]['ost'] = o2p.tile(
                                    [128, DIM], BF16, name="ost", tag="ost", bufs=4)
                            ost = holders[it]['ost']
                            dst = ost[:, oc * 512:(oc + 1) * 512]
                            if ib == NTB - 1:
                                nc.scalar.activation(dst, opps[:], AF.Identity)
                            else:
                                nc.vector.tensor_copy(dst, opps[:])
                            if oc == 1:
                                nc.sync.dma_start(
                                    out[i0 + it * 128:i0 + (it + 1) * 128, :],
                                    ost[:])
                        yield unit

            def phase1A_units(tb, xh, xl):
                """Yield k/q projection steps (fp8-DR 3-term + cast)."""
                t0 = tb * TBS
                for wpair, dst in (((wkh_t, wkl_t), kT), ((wqh_t, wql_t), qT)):
                    for ft in range(2):
                        holder = {}

                        def unit_a(wpair=wpair, ft=ft, holder=holder):
                            pps = ps.tile([128, TBS], F32, name="pps", tag="ps", bufs=2)
                            holder['pps'] = pps
                            for cp_ in range(4):
                                dr3(pps[:, 0:256], wpair, (xh, xl),
                                    lambda t, c=cp_, f=ft: t[:, 2 * c:2 * c + 2, f * 128:(f + 1) * 128],
                                    lambda t, c=cp_: t[:, 2 * c:2 * c + 2, 0:256],
                                    first=(cp_ == 0), last=(cp_ == 3))

                        def unit_b(wpair=wpair, dst=dst, ft=ft, holder=holder):
                            pps = holder['pps']
                            for cp_ in range(4):
                                dr3(pps[:, 256:512], wpair, (xh, xl),
                                    lambda t, c=cp_, f=ft: t[:, 2 * c:2 * c + 2, f * 128:(f + 1) * 128],
                                    lambda t, c=cp_: t[:, 2 * c:2 * c + 2, 256:512],
                                    first=(cp_ == 0), last=(cp_ == 3))
                            nc.vector.tensor_mul(dst[ft][:, t0:t0 + TBS], pps[:], s_b[tb][:])

                        yield unit_a
                        yield unit_b

            def attention(ib, fillers=()):
                """S/exp/PV over a unified (m, jt) stream with a lag-1
                pipeline; exact causal widths; exp covers both heads of a
                pair in one ACT op. Filler units (outproj of ib-1, k/q proj
                of ib+1) interleave between steps to hide exp latency, and
                the next pair's S matmuls run while the previous pair's
                normalization chain drains (o_ps reuse is gated on it)."""
                fill = list(fillers)
                fi = [0]
                i0 = ib * TBS
                njt = 4 * ib + 4
                nsteps = 2 * njt
                # finish the last window's fillers a few steps early so their
                # copies clear DVE before the tail normalization chain
                eff = nsteps + 3 if ib == NTB - 1 else nsteps

                def pop_fill(step):
                    # pace units evenly across the step stream
                    want = (len(fill) * (step + 1)) // eff
                    while fi[0] < min(want, len(fill)):
                        fill[fi[0]]()
                        fi[0] += 1
                o_ps = {}

                def emit_S(m, jt):
                    sft = jt * 128 - i0
                    diag = sft >= 0
                    off = max(sft, 0)
                    w = TBS - off
                    sp = ps.tile([128, 2, TBS], F32, name="sp", tag="sp2", bufs=2)
                    mw = min(w, 128)   # mask only touches cols with rows j>c
                    for h2 in range(2):
                        lo = h2 * 64
                        nc.tensor.matmul(sp[:, h2, off:],
                                         kT[m][lo:lo + 64, jt * 128:(jt + 1) * 128],
                                         qT[m][lo:lo + 64, i0 + off:i0 + TBS],
                                         start=True, stop=True)
                        if diag:
                            nc.tensor.matmul(sp[:, h2, off:off + mw], tri8_t[:],
                                             sel8_t[:, :, 0:mw],
                                             start=False, stop=True, perf_mode=DR,
                                             skip_group_check=True)
                    pT_ = pp.tile([128, 2, TBS], BF16, name="pT", tag="pT", bufs=6)
                    nc.scalar.activation(pT_[:, :, off:], sp[:, :, off:], AF.Exp)
                    return m, jt, pT_, off, w

                def emit_PV(rec):
                    m, jt, pT_, off, w = rec
                    if m not in o_ps:
                        o_ps[m] = [ps.tile([128, TBS], F32, name=f"o{m}_{h2}",
                                           tag="ops", bufs=2) for h2 in range(2)]
                    for h2 in range(2):
                        nc.tensor.matmul(o_ps[m][h2][0:DHEAD + 1, off:],
                                         v_sb[:, jt, 2 * m + h2, :],
                                         pT_[:, h2, off:],
                                         start=(jt == 0), stop=(jt == njt - 1))
                    if jt == njt - 1:
                        norm_pair(ib, m, o_ps[m],
                                  tail=(ib == NTB - 1 and m == 1))

                steps = [(m, jt) for m in range(2) for jt in range(njt)]
                prev = None
                for si, (m, jt) in enumerate(steps):
                    cur = emit_S(m, jt)
                    if prev is not None:
                        emit_PV(prev)
                    pop_fill(si)
                    prev = cur
                emit_PV(prev)
                while fi[0] < len(fill):
                    fill[fi[0]]()
                    fi[0] += 1

            def mark(name):
                _SECTIONS.append((name, nc.next_id()))

            xcur = (xh0, xl0, xq0)
            xnext = None
            deferred = []
            mark("rms_chain(0)")
            rms_chain(0, xq0)
            mark("phase1A(0)")
            for u in phase1A_units(0, xh0, xl0):
                u()
            for tb in range(NTB):
                xh, xl, xq = xcur
                fillers = list(deferred)
                deferred = []
                if tb > 0:
                    units = list(outproj_units(tb - 1))
                    if tb == 1:
                        fillers.extend(units[:2])
                        deferred = units[2:]
                    elif tb == 2:
                        # defer outproj(1) into attention(3), which
                        # has no projection fillers of its own
                        deferred = units
                    else:
                        fillers.extend(units)
                if tb + 1 < NTB:
                    t0n = (tb + 1) * TBS
                    mark(f"xprefetch({tb + 1})")
                    xqn = xp.tile([128, 8, TBS], F8H, name="xqn", tag="xq")
                    nc.sync.dma_start(xqn[:], xq8[:, :, t0n:t0n + TBS])
                    xtn = xp.tile([128, 2, 8, TBS], mybir.dt.uint8, name="xtn", tag="xhl")
                    nc.sync.dma_start(xtn[:].rearrange("p a c t -> p a (c t)"),
                                      xhl[:, :, tb + 1, :, :].rearrange("p a c t -> p a (c t)"))
                    xnext = (xtn[:, 0, :, :].bitcast(F8H),
                             xtn[:, 1, :, :].bitcast(F8L), xqn)
                if tb == 1:
                    nc.sync.dma_start(wo_t[:], wo[:])
                p1b = list(phase1B_units(tb, xh, xl))
                if tb < 2:
                    mark(f"phase1B({tb})")
                    for u in p1b:
                        u()
                    p1b = []
                if tb + 1 < NTB:
                    mark(f"rms_chain({tb + 1})")
                    rms_chain(tb + 1, xnext[2])
                    fillers = p1b + list(
                        phase1A_units(tb + 1, xnext[0], xnext[1])) + fillers
                else:
                    fillers = p1b + fillers
                mark(f"attention({tb})")
                attention(tb, fillers)
                xcur = xnext
            mark(f"outproj({NTB - 1})")
            for u in outproj_units(NTB - 1):
                u()
            mark("end")
    nc.finalize()
    return nc


_NC = None
_SECTIONS = []


def _get_nc():
    global _NC
    if _NC is None:
        _NC = _build()
    return _NC


def _consts():
    # tri8: [r, 0, j] = -60 if j > r (strict upper in (r, j)); block 1 zero
    tri = np.zeros((128, 2, 128), np.float32)
    tri[:, 0, :] = np.triu(np.full((128, 128), -60.0, np.float32), 1)
    # sel8: [r, 0, c] = 1 iff r == c (identity window); block 1 zero
    sel = np.zeros((128, 2, TBS), np.float32)
    sel[:, 0, 0:128] = np.eye(128, dtype=np.float32)
    ones = np.ones((128, 2, 128), np.float32)
    idn = np.eye(128, dtype=np.float32)
    c8 = np.concatenate([
        ones.astype(NP_F8H).view(np.uint8),
        tri.astype(NP_F8H).view(np.uint8),
        sel.astype(NP_F8H).view(np.uint8),
    ], axis=2)
    return dict(
        c8blob=np.ascontiguousarray(c8),
        idnb=idn.astype(np.float32),
    )


_LAST_RESULTS = None


def kernel(x, mask, g, w_qkv, w_out, _trace=False, _trace_kwargs=None):
    global _LAST_RESULTS
    x = np.asarray(x, np.float32)
    mask_f = np.asarray(mask).astype(np.float32)
    g = np.asarray(g, np.float32)
    w_qkv = np.asarray(w_qkv, np.float32)
    w_out = np.asarray(w_out, np.float32)

    nc = _get_nc()
    consts = _consts()

    # per-batch x marshalling (shared across the 4 head-group cores)
    xb_maps = []
    for b in range(B):
        xT = np.ascontiguousarray(x[b].T).reshape(8, 128, N).transpose(1, 0, 2)
        xh = xT.astype(NP_F8H)
        xl = (xT - xh.astype(np.float32)).astype(NP_F8L)
        xhl_b = np.ascontiguousarray(
            np.stack([xh.view(np.uint8), xl.view(np.uint8)], axis=1)
            .reshape(128, 2, 8, NTB, TBS).transpose(0, 1, 3, 2, 4))
        xq = (xT * xT).astype(NP_F8H)
        maskv_b = np.ascontiguousarray(mask_f[b].reshape(NJT, 128).T)
        xb_maps.append(dict(xhl=xhl_b, xq8=xq, maskv=maskv_b))

    def wprep(wslice, scale):
        # [1024, GF] -> [128, 8, GF], fold g and WS (and scale), hi+lo fp8
        wf = (wslice * g[:, None] * (WS * scale)).reshape(8, 128, GF).transpose(1, 0, 2)
        wh = wf.astype(NP_F8H)
        wl = (wf - wh.astype(np.float32)).astype(NP_F8L)
        return wh, wl

    in_maps = []
    for b in range(B):
        for hg in range(4):
            sl = slice(hg * GF, (hg + 1) * GF)
            wqh, wql = wprep(w_qkv[:, 0 * 1024:][:, sl], DHEAD ** -0.5)
            wkh, wkl = wprep(w_qkv[:, 1 * 1024:][:, sl], 1.0)
            wvh, wvl = wprep(w_qkv[:, 2 * 1024:][:, sl], 1.0)
            wb = np.ascontiguousarray(np.stack(
                [t.reshape(128, 8 * GF).view(np.uint8)
                 for t in (wkh, wkl, wqh, wql, wvh, wvl)], axis=1))
            wo_c = np.ascontiguousarray(
                w_out[sl, :].reshape(2, 128, DIM).transpose(1, 0, 2)).astype(NP_BF16)
            in_maps.append(dict(
                wblob=wb, wo=wo_c, **xb_maps[b], **consts,
            ))
    kwargs = {}
    if _trace:
        kwargs["trace"] = True
        kwargs.update(_trace_kwargs or {})
    res = run_bass_kernel_spmd(nc, in_maps, core_ids=list(range(NCORES)), **kwargs)
    _LAST_RESULTS = res
    outv = np.zeros((B, N, DIM), np.float32)
    for b in range(B):
        for hg in range(4):
            outv[b] += res.results[b * 4 + hg]["out"].astype(np.float32)
    return outv



# revision 5
# speedup vs baseline: 1.0025x; 1.0025x over previous
"""Trainium2 Bass kernel v2 for nn_Attention_9921374454177.

RMSNorm -> QKV proj -> 16-head causal attention -> out proj.
Sharding: 8 cores = 2 batches x 4 head-groups; host sums the 4 partial
out-projections per batch.

v2 changes vs baseline (tuned against the TimelineSim cost model):
  - QKV projections run as fp8 DoubleRow matmuls with hi+lo error
    compensation: x and w are shipped as (e4m3 hi, e5m2 residual) pairs and
    the projection accumulates the 3 significant cross terms
    (hi*hi + hi*lo + lo*hi) in one PSUM group. 0.75x the PE cost of bf16 at
    ~bf16 accuracy. Weights are pre-scaled x16 on host (fp8 subnormal
    avoidance); the descale rides the existing s_b/s_pp token-scale folds.
  - sum-of-squares ships x^2 as e4m3 from host and reduces it with an fp8
    DoubleRow ones-matmul (4x cheaper than the bf16 ones-trick, and no ACT
    Square pass at all).
  - S, PV, and the out-projection run in bf16 (fp8 at these sites measures
    1.8-3.5e-2 final rel err -- over the 2e-2 gate), with exact causal
    widths per j-tile (bf16 has no 256-min-width penalty like fp32r).
  - causal mask added by the PE as an fp8 DoubleRow rank-structured matmul
    (strict-lower-tri(-60) @ identity window) into the same PSUM group.
  - mask folded into the v token scale and the l ones-column (exact masking
    through the PV matmul) -> exp needs no bias/scale AP at all; k-norm
    scale folded into the k PSUM->bf16 cast instead of the exp scale.
  - exp batched over both heads of a pair (one ACT op per (m, jt)).
  - attention emits one unified (m, jt) stream with a lag-1 exp pipeline;
    out-projection of earlier blocks and k/q/v projection of the next block
    are interleaved as paced filler units between steps (with deferral so
    the later, larger i-batches get enough independent PE work), and the
    next pair's S matmuls overlap the previous pair's normalization chain.
  - causal-mask add truncated to the 128 columns it can affect; startup
    DMAs blob-packed (u8 + bitcast views) and ordered by first consumer.
  - note: GPSIMD ops and the custom-DVE reciprocal cannot read PSUM on
    hardware -- every PSUM consumer here is a plain DVE/ACT op or the PE.
"""
import numpy as np
import ml_dtypes

import concourse.bacc as bacc
import concourse.mybir as mybir
import concourse.tile as tile
from concourse.bass_utils import run_bass_kernel_spmd

F32 = mybir.dt.float32
BF16 = mybir.dt.bfloat16
F8H = mybir.dt.float8e4   # e4m3
F8L = mybir.dt.float8e5   # e5m2
AF = mybir.ActivationFunctionType
OP = mybir.AluOpType
DR = mybir.MatmulPerfMode.DoubleRow

NP_F8H = ml_dtypes.float8_e4m3
NP_F8L = ml_dtypes.float8_e5m2
NP_BF16 = ml_dtypes.bfloat16

B, N, DIM = 2, 2048, 1024
HEADS, DHEAD = 16, 64
GH = 4                 # heads per core
GF = GH * DHEAD        # 256 features per core
NCORES = 8
TBS = 512              # token block size
NTB = N // TBS         # 4
NJT = N // 128         # 16 j-tiles
WS = 16.0              # host weight pre-scale (fp8 subnormal avoidance)
LN2 = float(np.log(2.0))   # exp bias: s_b = 2*ss^-1/2 = (s/16 with s=32*ss^-1/2)

_COMBINED_ACT_SET = "natural_log_exp_and_others"


class _Bacc(bacc.Bacc):
    """Pin the ln+exp combined ACT table so Ln/Exp share one table load."""

    def insert_act_table_loads(self):
        import bass_rust as _bass_rust
        from concourse.hw_specs import get_activation_tables

        has_activation = any(
            isinstance(i, mybir.InstActivation)
            for b in self.main_func.blocks
            for i in b.instructions
        )
        if not has_activation:
            return
        tables = [
            (name, funcs if name == _COMBINED_ACT_SET else set())
            for name, funcs in get_activation_tables(self.m.arch).items()
        ]
        _bass_rust.insert_act_table_loads(self, tables)


def _build():
    nc = _Bacc()
    U8 = mybir.dt.uint8
    xhl = nc.declare_dram_parameter("xhl", [128, 2, NTB, 8, TBS], U8, isOutput=False)
    xq8 = nc.declare_dram_parameter("xq8", [128, 8, N], F8H, isOutput=False)
    wblob = nc.declare_dram_parameter("wblob", [128, 6, 8 * GF], U8, isOutput=False)
    wo = nc.declare_dram_parameter("wo", [128, 2, DIM], BF16, isOutput=False)
    c8blob = nc.declare_dram_parameter("c8blob", [128, 2, 640], U8, isOutput=False)
    maskv = nc.declare_dram_parameter("maskv", [128, NJT], F32, isOutput=False)
    idnb = nc.declare_dram_parameter("idnb", [128, 128], F32, isOutput=False)
    out = nc.declare_dram_parameter("out", [N, DIM], BF16, isOutput=True)

    with tile.TileContext(nc) as tc:
        with (
            tc.tile_pool(name="const", bufs=1) as cp,
            tc.tile_pool(name="xsl", bufs=2) as xp,
            tc.tile_pool(name="sm", bufs=1) as smp,
            tc.tile_pool(name="pTp", bufs=4) as pp,
            tc.tile_pool(name="lstp", bufs=1) as lp,
            tc.tile_pool(name="bcp", bufs=1) as bp,
            tc.tile_pool(name="O2p", bufs=4) as o2p,
            tc.tile_pool(name="ps", bufs=8, space="PSUM") as ps,
        ):
            # ---- startup: touch ACT so the single activation-table load
            # runs during the prologue DMAs, off the first Ln's critical path
            actwarm = cp.tile([128, 1], F32, name="actwarm")
            nc.vector.memset(actwarm[:], 1.0)
            nc.scalar.activation(actwarm[:], actwarm[:], AF.Exp)
            # ss-DR ones come from an on-chip memset (no DMA on the
            # first-matmul critical path)
            ones_t = cp.tile([128, 2, 128], F8H, name="ones8_t")
            nc.vector.memset(ones_t[:], 1.0)

            # ---- startup DMAs, ordered by first consumer: xq (ss-DR),
            # wk + x chunk-pair pieces (k-proj), then the rest.
            xq0 = xp.tile([128, 8, TBS], F8H, name="xq0", tag="xq")
            nc.sync.dma_start(xq0[:], xq8[:, :, 0:TBS])
            wb_t = cp.tile([128, 6, 8 * GF], mybir.dt.uint8, name="wb_t")
            nc.sync.dma_start(wb_t[:, 0:2, :], wblob[:, 0:2, :])

            def wview(i, dt_):
                return wb_t[:, i, :].rearrange("p (c f) -> p c f", f=GF).bitcast(dt_)

            wkh_t, wkl_t = wview(0, F8H), wview(1, F8L)
            wqh_t, wql_t = wview(2, F8H), wview(3, F8L)
            wvh_t, wvl_t = wview(4, F8H), wview(5, F8L)

            # x hi+lo for block 0 in 4 chunk-pair pieces ordered so the first
            # k-projection chunk-pair groups unblock incrementally
            xt0 = xp.tile([128, 2, 8, TBS], mybir.dt.uint8, name="xt0", tag="xhl")
            for c0, hl in ((0, 0), (0, 1), (4, 0), (4, 1)):
                nc.sync.dma_start(xt0[:, hl, c0:c0 + 4, :],
                                  xhl[:, hl, 0, c0:c0 + 4, :])
            xh0 = xt0[:, 0, :, :].bitcast(F8H)
            xl0 = xt0[:, 1, :, :].bitcast(F8L)

            nc.sync.dma_start(wb_t[:, 2:4, :], wblob[:, 2:4, :])
            nc.sync.dma_start(wb_t[:, 4:6, :], wblob[:, 4:6, :])
            c8_t = cp.tile([128, 2, 640], mybir.dt.uint8, name="c8_t")
            nc.sync.dma_start(c8_t[:], c8blob[:])
            tri8_t = c8_t[:, :, 0:128].bitcast(F8H)
            sel8_t = c8_t[:, :, 128:640].bitcast(F8H)
            maskv_t = cp.tile([128, NJT], F32, name="maskv_t")
            nc.sync.dma_start(maskv_t[:], maskv[:])
            idnb_t = cp.tile([128, 128], F32, name="idnb_t")
            nc.sync.dma_start(idnb_t[:], idnb[:])
            wo_t = cp.tile([128, 2, DIM], BF16, name="wo_t")

            ln2_t = cp.tile([128, 1], F32, name="ln2_t")
            nc.vector.memset(ln2_t[:], LN2)
            ones64_t = cp.tile([128, 64], F32, name="ones64_t")
            nc.vector.memset(ones64_t[:], 1.0)

            # ---- persistent activation tensors ----
            v_sb = cp.tile([128, NJT, GH, DHEAD + 1], BF16, name="v_sb")
            # l ones-column <- mask (exact masking through the PV matmul)
            for h in range(GH):
                nc.vector.tensor_copy(
                    v_sb[:, :, h, DHEAD:DHEAD + 1].rearrange("p a b -> p (a b)"),
                    maskv_t[:])
            kT = [cp.tile([128, N], BF16, name=f"kT{ft}") for ft in range(2)]
            qT = [cp.tile([128, N], BF16, name=f"qT{ft}") for ft in range(2)]
            s_b = [cp.tile([128, TBS], F32, name=f"s_b{tb}") for tb in range(NTB)]
            s_pv = cp.tile([128, NJT], F32, name="s_pv")

            o2_of = {}

            def dr3(out_ap, lhs_hl, rhs_hl, lhs_sl, rhs_sl, first, last):
                """One 3-term hi+lo DoubleRow contraction step into out_ap.
                lhs_hl/rhs_hl = (hi_tile, lo_tile); *_sl = slicer functions."""
                lh, ll = lhs_hl
                rh, rl = rhs_hl
                terms = [(lh, rh), (lh, rl), (ll, rh)]
                for i, (lt, rt) in enumerate(terms):
                    nc.tensor.matmul(out_ap, lhs_sl(lt), rhs_sl(rt),
                                     start=(first and i == 0),
                                     stop=(last and i == 2),
                                     perf_mode=DR)

            def rms_chain(tb, xq):
                """sum-of-squares (fp8-DR ones-trick) -> s_b[tb] = 2*ss^-0.5."""
                ss_ps = ps.tile([128, TBS], F32, name="ss_ps", tag="ps", bufs=2)
                for th in range(2):
                    for cp_ in range(4):
                        nc.tensor.matmul(
                            ss_ps[:, th * 256:(th + 1) * 256],
                            ones_t[:],
                            xq[:, 2 * cp_:2 * cp_ + 2, th * 256:(th + 1) * 256],
                            start=(cp_ == 0), stop=(cp_ == 3), perf_mode=DR)
                lnt = smp.tile([128, TBS], F32, name="lnt", tag="lnt")
                nc.scalar.activation(lnt[:], ss_ps[:], AF.Ln)
                s0 = smp.tile([128, TBS], F32, name="s0", tag="s0")
                nc.scalar.activation(s0[:], lnt[:], AF.Exp, scale=-0.5, bias=ln2_t[:])
                u_t = smp.tile([128, TBS], F32, name="u_t", tag="u_t")
                nc.vector.tensor_mul(u_t[:], s0[:], s0[:])
                w_t = smp.tile([128, TBS], F32, name="w_t", tag="w_t")
                nc.vector.tensor_mul(w_t[:], u_t[:], ss_ps[:])
                nc.vector.tensor_scalar(w_t[:], w_t[:], -0.125, 1.5, OP.mult, OP.add)
                nc.vector.tensor_mul(s_b[tb][:], s0[:], w_t[:])

            def phase1B_units(tb, xh, xl):
                """Yield v-projection steps for block tb (transpose + scale
                fold first, then 4 fp8-DR 3-term quarter-blocks)."""
                t0 = tb * TBS

                def unit_t():
                    tps = ps.tile([128, TBS], F32, name="tps", tag="ps", bufs=2)
                    for j in range(4):
                        nc.tensor.transpose(tps[:, j * 128:(j + 1) * 128],
                                            s_b[tb][:, j * 128:(j + 1) * 128], idnb_t[:])
                    s_pp_blk = smp.tile([128, 4], F32, name="s_pp_blk", tag="spb")
                    nc.vector.tensor_copy(
                        s_pp_blk[:],
                        tps[:].rearrange("p (j q) -> p j q", q=128)[:, :, 0:1]
                            .rearrange("p j q -> p (j q)"))
                    nc.vector.tensor_mul(s_pv[:, tb * 4:(tb + 1) * 4], s_pp_blk[:],
                                         maskv_t[:, tb * 4:(tb + 1) * 4])

                yield unit_t
                for tsub in range(4):
                    def unit_v(tsub=tsub):
                        vps = ps.tile([128, GF], F32, name="vps", tag="ps", bufs=2)
                        for cp_ in range(4):
                            dr3(vps[:],
                                (xh, xl), (wvh_t, wvl_t),
                                lambda t, c=cp_, ts=tsub: t[:, 2 * c:2 * c + 2, ts * 128:(ts + 1) * 128],
                                lambda t, c=cp_: t[:, 2 * c:2 * c + 2, :],
                                first=(cp_ == 0), last=(cp_ == 3))
                        t_idx = tb * 4 + tsub
                        nc.vector.tensor_scalar_mul(
                            v_sb[:, t_idx, :, 0:DHEAD],
                            vps[:].rearrange("p (h d) -> p h d", d=DHEAD),
                            s_pv[:, t_idx:t_idx + 1])

                    yield unit_v

            def norm_pair(ib, m, o_ps, tail):
                """1/l + normalization for head pair m of i-batch ib."""
                O2m = o2p.tile([128, TBS], BF16, name=f"O2_{m}", tag="O2", bufs=6)
                o2_of[(ib, m)] = O2m
                lst = lp.tile([1, 2 * TBS], F32, name="lst", tag="lst", bufs=2)
                rcl = lp.tile([1, 2 * TBS], F32, name="rcl", tag="rcl", bufs=2)
                if tail:
                    # chunked 128-col chains (copy on idle ACT, recip+mul on
                    # DVE, bcast on Pool) so outproj's stationary loads
                    # unblock it-by-it right behind the last PV
                    bchs = [bp.tile([64, TBS], F32, name=f"bch{h2}", tag="bch",
                                    bufs=2) for h2 in range(2)]
                    for hf in range(2):
                        cs = slice(hf * 256, (hf + 1) * 256)
                        for h2 in range(2):
                            lsl = slice(h2 * TBS + hf * 256, h2 * TBS + (hf + 1) * 256)
                            nc.scalar.activation(lst[0:1, lsl],
                                                 o_ps[h2][64:65, cs], AF.Identity)
                            nc.vector.reciprocal_approx_fast(
                                out=rcl[0:1, lsl], in_=lst[0:1, lsl])
                            nc.gpsimd.partition_broadcast(
                                bchs[h2][:, cs], rcl[0:1, lsl])
                            nc.vector.tensor_mul(
                                O2m[h2 * 64:(h2 + 1) * 64, cs],
                                o_ps[h2][0:DHEAD, cs], bchs[h2][:, cs])
                else:
                    for h2 in range(2):
                        nc.vector.tensor_copy(lst[0:1, h2 * TBS:(h2 + 1) * TBS],
                                              o_ps[h2][64:65, :])
                        nc.vector.reciprocal_approx_fast(
                            out=rcl[0:1, h2 * TBS:(h2 + 1) * TBS],
                            in_=lst[0:1, h2 * TBS:(h2 + 1) * TBS])
                        bch = bp.tile([64, TBS], F32, name=f"bch{h2}", tag="bch", bufs=2)
                        nc.gpsimd.partition_broadcast(
                            bch[:], rcl[0:1, h2 * TBS:(h2 + 1) * TBS])
                        nc.vector.tensor_mul(O2m[h2 * 64:(h2 + 1) * 64, :],
                                             o_ps[h2][0:DHEAD, :], bch[:])

            def outproj_units(ib):
                """Yield fine-grained outproj steps; one out-DMA per
                128-token row tile (both oc halves share one ost tile)."""
                i0 = ib * TBS
                holders = [dict() for _ in range(4)]
                for it in range(4):
                    for oc in range(2):
                        def unit(it=it, oc=oc):
                            opps = ps.tile([128, TBS], F32, name="opps", tag="ps", bufs=2)
                            for m in range(2):
                                nc.tensor.matmul(opps[:],
                                                 o2_of[(ib, m)][:, it * 128:(it + 1) * 128],
                                                 wo_t[:, m, oc * 512:(oc + 1) * 512],
                                                 start=(m == 0), stop=(m == 1))
                            if oc == 0:
                                holders[it]['ost'] = o2p.tile(
                                    [128, DIM], BF16, name="ost", tag="ost", bufs=4)
                            ost = holders[it]['ost']
                            dst = ost[:, oc * 512:(oc + 1) * 512]
                            if ib == NTB - 1:
                                # drain fast: alternate DVE/ACT for the copies
                                # and DMA each 512-col half as soon as written
                                if oc == 0:
                                    nc.vector.tensor_copy(dst, opps[:])
                                else:
                                    nc.scalar.activation(dst, opps[:], AF.Identity)
                                nc.sync.dma_start(
                                    out[i0 + it * 128:i0 + (it + 1) * 128,
                                        oc * 512:(oc + 1) * 512],
                                    dst)
                            else:
                                nc.vector.tensor_copy(dst, opps[:])
                                if oc == 1:
                                    nc.sync.dma_start(
                                        out[i0 + it * 128:i0 + (it + 1) * 128, :],
                                        ost[:])
                        yield unit

            def phase1A_units(tb, xh, xl):
                """Yield k/q projection steps (fp8-DR 3-term + cast)."""
                t0 = tb * TBS
                for wpair, dst in (((wkh_t, wkl_t), kT), ((wqh_t, wql_t), qT)):
                    for ft in range(2):
                        holder = {}

                        def unit_a(wpair=wpair, ft=ft, holder=holder):
                            pps = ps.tile([128, TBS], F32, name="pps", tag="ps", bufs=2)
                            holder['pps'] = pps
                            for cp_ in range(4):
                                dr3(pps[:, 0:256], wpair, (xh, xl),
                                    lambda t, c=cp_, f=ft: t[:, 2 * c:2 * c + 2, f * 128:(f + 1) * 128],
                                    lambda t, c=cp_: t[:, 2 * c:2 * c + 2, 0:256],
                                    first=(cp_ == 0), last=(cp_ == 3))

                        def unit_b(wpair=wpair, dst=dst, ft=ft, holder=holder):
                            pps = holder['pps']
                            for cp_ in range(4):
                                dr3(pps[:, 256:512], wpair, (xh, xl),
                                    lambda t, c=cp_, f=ft: t[:, 2 * c:2 * c + 2, f * 128:(f + 1) * 128],
                                    lambda t, c=cp_: t[:, 2 * c:2 * c + 2, 256:512],
                                    first=(cp_ == 0), last=(cp_ == 3))
                            nc.vector.tensor_mul(dst[ft][:, t0:t0 + TBS], pps[:], s_b[tb][:])

                        yield unit_a
                        yield unit_b

            def attention(ib, fillers=()):
                """S/exp/PV over a unified (m, jt) stream with a lag-1
                pipeline; exact causal widths; exp covers both heads of a
                pair in one ACT op. Filler units (outproj of ib-1, k/q proj
                of ib+1) interleave between steps to hide exp latency, and
                the next pair's S matmuls run while the previous pair's
                normalization chain drains (o_ps reuse is gated on it)."""
                fill = list(fillers)
                fi = [0]
                i0 = ib * TBS
                njt = 4 * ib + 4
                nsteps = 2 * njt
                # finish the last window's fillers a few steps early so their
                # copies clear DVE before the tail normalization chain
                eff = nsteps + 3 if ib == NTB - 1 else nsteps

                def pop_fill(step):
                    # pace units evenly across the step stream
                    want = (len(fill) * (step + 1)) // eff
                    while fi[0] < min(want, len(fill)):
                        fill[fi[0]]()
                        fi[0] += 1
                o_ps = {}

                def emit_S(m, jt):
                    sft = jt * 128 - i0
                    diag = sft >= 0
                    off = max(sft, 0)
                    w = TBS - off
                    sp = ps.tile([128, 2, TBS], F32, name="sp", tag="sp2", bufs=2)
                    mw = min(w, 128)   # mask only touches cols with rows j>c
                    for h2 in range(2):
                        lo = h2 * 64
                        nc.tensor.matmul(sp[:, h2, off:],
                                         kT[m][lo:lo + 64, jt * 128:(jt + 1) * 128],
                                         qT[m][lo:lo + 64, i0 + off:i0 + TBS],
                                         start=True, stop=True)
                        if diag:
                            nc.tensor.matmul(sp[:, h2, off:off + mw], tri8_t[:],
                                             sel8_t[:, :, 0:mw],
                                             start=False, stop=True, perf_mode=DR,
                                             skip_group_check=True)
                    pT_ = pp.tile([128, 2, TBS], BF16, name="pT", tag="pT", bufs=6)
                    nc.scalar.activation(pT_[:, :, off:], sp[:, :, off:], AF.Exp)
                    return m, jt, pT_, off, w

                def emit_PV(rec):
                    m, jt, pT_, off, w = rec
                    if m not in o_ps:
                        o_ps[m] = [ps.tile([128, TBS], F32, name=f"o{m}_{h2}",
                                           tag="ops", bufs=2) for h2 in range(2)]
                    for h2 in range(2):
                        nc.tensor.matmul(o_ps[m][h2][0:DHEAD + 1, off:],
                                         v_sb[:, jt, 2 * m + h2, :],
                                         pT_[:, h2, off:],
                                         start=(jt == 0), stop=(jt == njt - 1))
                    if jt == njt - 1:
                        norm_pair(ib, m, o_ps[m],
                                  tail=(ib == NTB - 1 and m == 1))

                steps = [(m, jt) for m in range(2) for jt in range(njt)]
                prev = None
                for si, (m, jt) in enumerate(steps):
                    cur = emit_S(m, jt)
                    if prev is not None:
                        emit_PV(prev)
                    pop_fill(si)
                    prev = cur
                emit_PV(prev)
                while fi[0] < len(fill):
                    fill[fi[0]]()
                    fi[0] += 1

            def mark(name):
                _SECTIONS.append((name, nc.next_id()))

            xcur = (xh0, xl0, xq0)
            xnext = None
            deferred = []
            mark("rms_chain(0)")
            rms_chain(0, xq0)
            mark("phase1A(0)")
            for u in phase1A_units(0, xh0, xl0):
                u()
            for tb in range(NTB):
                xh, xl, xq = xcur
                fillers = list(deferred)
                deferred = []
                if tb > 0:
                    units = list(outproj_units(tb - 1))
                    if tb == 1:
                        fillers.extend(units[:2])
                        deferred = units[2:]
                    elif tb == 2:
                        # defer outproj(1) into attention(3), which
                        # has no projection fillers of its own
                        deferred = units
                    else:
                        fillers.extend(units)
                if tb + 1 < NTB:
                    t0n = (tb + 1) * TBS
                    mark(f"xprefetch({tb + 1})")
                    xqn = xp.tile([128, 8, TBS], F8H, name="xqn", tag="xq")
                    nc.sync.dma_start(xqn[:], xq8[:, :, t0n:t0n + TBS])
                    xtn = xp.tile([128, 2, 8, TBS], mybir.dt.uint8, name="xtn", tag="xhl")
                    nc.sync.dma_start(xtn[:].rearrange("p a c t -> p a (c t)"),
                                      xhl[:, :, tb + 1, :, :].rearrange("p a c t -> p a (c t)"))
                    xnext = (xtn[:, 0, :, :].bitcast(F8H),
                             xtn[:, 1, :, :].bitcast(F8L), xqn)
                if tb == 1:
                    nc.sync.dma_start(wo_t[:], wo[:])
                p1b = list(phase1B_units(tb, xh, xl))
                if tb < 2:
                    mark(f"phase1B({tb})")
                    for u in p1b:
                        u()
                    p1b = []
                if tb + 1 < NTB:
                    mark(f"rms_chain({tb + 1})")
                    rms_chain(tb + 1, xnext[2])
                    fillers = p1b + list(
                        phase1A_units(tb + 1, xnext[0], xnext[1])) + fillers
                else:
                    fillers = p1b + fillers
                mark(f"attention({tb})")
                attention(tb, fillers)
                xcur = xnext
            mark(f"outproj({NTB - 1})")
            for u in outproj_units(NTB - 1):
                u()
            mark("end")
    nc.finalize()
    return nc


_NC = None
_SECTIONS = []


def _get_nc():
    global _NC
    if _NC is None:
        _NC = _build()
    return _NC


def _consts():
    # tri8: [r, 0, j] = -60 if j > r (strict upper in (r, j)); block 1 zero
    tri = np.zeros((128, 2, 128), np.float32)
    tri[:, 0, :] = np.triu(np.full((128, 128), -60.0, np.float32), 1)
    # sel8: [r, 0, c] = 1 iff r == c (identity window); block 1 zero
    sel = np.zeros((128, 2, TBS), np.float32)
    sel[:, 0, 0:128] = np.eye(128, dtype=np.float32)
    idn = np.eye(128, dtype=np.float32)
    c8 = np.concatenate([
        tri.astype(NP_F8H).view(np.uint8),
        sel.astype(NP_F8H).view(np.uint8),
    ], axis=2)
    return dict(
        c8blob=np.ascontiguousarray(c8),
        idnb=idn.astype(np.float32),
    )


_LAST_RESULTS = None


def kernel(x, mask, g, w_qkv, w_out, _trace=False, _trace_kwargs=None):
    global _LAST_RESULTS
    x = np.asarray(x, np.float32)
    mask_f = np.asarray(mask).astype(np.float32)
    g = np.asarray(g, np.float32)
    w_qkv = np.asarray(w_qkv, np.float32)
    w_out = np.asarray(w_out, np.float32)

    nc = _get_nc()
    consts = _consts()

    # per-batch x marshalling (shared across the 4 head-group cores)
    xb_maps = []
    for b in range(B):
        xT = np.ascontiguousarray(x[b].T).reshape(8, 128, N).transpose(1, 0, 2)
        xh = xT.astype(NP_F8H)
        xl = (xT - xh.astype(np.float32)).astype(NP_F8L)
        xhl_b = np.ascontiguousarray(
            np.stack([xh.view(np.uint8), xl.view(np.uint8)], axis=1)
            .reshape(128, 2, 8, NTB, TBS).transpose(0, 1, 3, 2, 4))
        xq = (xT * xT).astype(NP_F8H)
        maskv_b = np.ascontiguousarray(mask_f[b].reshape(NJT, 128).T)
        xb_maps.append(dict(xhl=xhl_b, xq8=xq, maskv=maskv_b))

    def wprep(wslice, scale):
        # [1024, GF] -> [128, 8, GF], fold g and WS (and scale), hi+lo fp8
        wf = (wslice * g[:, None] * (WS * scale)).reshape(8, 128, GF).transpose(1, 0, 2)
        wh = wf.astype(NP_F8H)
        wl = (wf - wh.astype(np.float32)).astype(NP_F8L)
        return wh, wl

    in_maps = []
    for b in range(B):
        for hg in range(4):
            sl = slice(hg * GF, (hg + 1) * GF)
            wqh, wql = wprep(w_qkv[:, 0 * 1024:][:, sl], DHEAD ** -0.5)
            wkh, wkl = wprep(w_qkv[:, 1 * 1024:][:, sl], 1.0)
            wvh, wvl = wprep(w_qkv[:, 2 * 1024:][:, sl], 1.0)
            wb = np.ascontiguousarray(np.stack(
                [t.reshape(128, 8 * GF).view(np.uint8)
                 for t in (wkh, wkl, wqh, wql, wvh, wvl)], axis=1))
            wo_c = np.ascontiguousarray(
                w_out[sl, :].reshape(2, 128, DIM).transpose(1, 0, 2)).astype(NP_BF16)
            in_maps.append(dict(
                wblob=wb, wo=wo_c, **xb_maps[b], **consts,
            ))
    kwargs = {}
    if _trace:
        kwargs["trace"] = True
        kwargs.update(_trace_kwargs or {})
    res = run_bass_kernel_spmd(nc, in_maps, core_ids=list(range(NCORES)), **kwargs)
    _LAST_RESULTS = res
    outv = np.zeros((B, N, DIM), np.float32)
    for b in range(B):
        for hg in range(4):
            outv[b] += res.results[b * 4 + hg]["out"].astype(np.float32)
    return outv

